# revision 1
# baseline (speedup 1.0000x reference)
"""Trainium2 Bass kernel for nn_BondPredictor (gnn_message_passing).

Computes, for each batch b:
    A      = hidden_states[b][clip(atom_indices[b])]          # [256, 512] gather
    pair   = concat(A[i]+A[j], |A[i]-A[j]|)                   # [256,256,1024]
    h      = gelu(pair @ W1 + b1)                             # [256,256,512]
    logits = h @ W2 + b2  -> [7, 256, 256], diagonal = -10000

Sharding: 8 cores = 2 batches x 4 row-blocks of 64 rows. Each core's atom
axis is ROLLED by -64*(c%4) so every core computes rows 0..63 of its rolled
grid with an identical program (pure SPMD); the host un-rolls the output
columns when unsharding.

Symmetry: pair(i,j) == pair(j,i) exactly, so each row only computes the
cyclic half-window of columns (j-i) mod 256 in [0,128] (129 values: 128 in
the quad loop + 1 antipodal column); the host mirrors offsets 129..255 from
the transpose during unshard. This halves all per-pair compute.

Algebraic split: (A[i]+A[j]) @ W1a = P[i] + P[j] with P = A @ W1a computed
once per core -> only the |A[i]-A[j]| half needs the big per-pair matmul,
and with |d| = 2*relu(d) - d the per-pair contraction uses relu features
(a valid subtract+max DVE dual-op) while the linear -d part folds into
Q = A @ W1b. The per-pair rank-one terms (P-Q)[j] + (P+Q+b1)[i] enter PSUM
through an identity-weight matmul; b2 rides the PSUM->SBUF copy; the
diagonal -10000 fill is a single affine_select per row-quad on GPSIMD.
All matmuls bf16 with fp32 PSUM accumulation; exact erf-GELU on ScalarE.
"""

import sys

sys.path.insert(0, "/opt/trn_rl_repo")

import numpy as np
import ml_dtypes

B, T, D, N, C = 2, 1024, 512, 256, 7
NCORES = 8
RB = 4                # row-blocks per batch
NL = N // RB          # 64 rows per core
QR = 4                # rows per quad
NQ = NL // QR         # 16 quads
KC = D // 128         # 4 chunks of the 512-dim contraction
TC_ = T // 128        # 8 chunks of the sequence dim
TW = 128              # cols per row: cyclic offsets (j-i) mod N in [0,127];
                      # offset 128 is the extra antipodal column; offsets
                      # 129..255 come from the exact grid symmetry (mirror)
MASK_FILL = -10000.0

_CACHE = {}


def _build(reps=1):
    """Build + compile the per-core Bass program. Returns (nc, names)."""
    import concourse.bass as bass
    import concourse.bacc as bacc
    import concourse.tile as tile
    from concourse import mybir

    f32 = mybir.dt.float32
    bf16 = mybir.dt.bfloat16
    i32 = mybir.dt.int32
    Alu = mybir.AluOpType
    Act = mybir.ActivationFunctionType

    nc = bacc.Bacc("TRN2", target_bir_lowering=False, debug=False)

    h_d = nc.dram_tensor("h", [T, D], bf16, kind="ExternalInput")
    idx_d = nc.dram_tensor("idxf", [1, N], f32, kind="ExternalInput")
    w1_d = nc.dram_tensor("w1", [2 * D, D], bf16, kind="ExternalInput")
    w2_d = nc.dram_tensor("w2", [D, C], bf16, kind="ExternalInput")
    b1_d = nc.dram_tensor("b1", [D, 1], f32, kind="ExternalInput")
    b2_d = nc.dram_tensor("b2", [C, 1], f32, kind="ExternalInput")
    out_d = nc.dram_tensor("out", [C, NL, TW + 1], f32, kind="ExternalOutput")

    h_ap, idx_ap = h_d.ap(), idx_d.ap()
    w1_ap, w2_ap = w1_d.ap(), w2_d.ap()
    b1_ap, b2_ap, out_ap = b1_d.ap(), b2_d.ap(), out_d.ap()

    with tile.TileContext(nc) as tc:
        from contextlib import ExitStack

        with ExitStack() as ctx:
            const = ctx.enter_context(tc.tile_pool(name="const", bufs=1))
            wpool = ctx.enter_context(tc.tile_pool(name="w", bufs=1))
            gpool = ctx.enter_context(tc.tile_pool(name="g", bufs=2))
            work = ctx.enter_context(tc.tile_pool(name="work", bufs=4))
            opool = ctx.enter_context(tc.tile_pool(name="o", bufs=3))
            ph = ctx.enter_context(
                tc.tile_pool(name="ph", bufs=4, space=bass.MemorySpace.PSUM)
            )
            po = ctx.enter_context(
                tc.tile_pool(name="po", bufs=3, space=bass.MemorySpace.PSUM)
            )

            # ---- one-time constants (outside rep loop) ----
            ones1 = const.tile([1, 128], f32, tag="ones1")
            nc.vector.memset(ones1[:], 1.0)
            onesq = const.tile([128, 128], bf16, tag="onesq")
            nc.vector.memset(onesq[:], 1.0)
            ident = const.tile([128, 128], bf16, tag="ident")
            # iota[p,f] = p - f -> ==0 on the diagonal
            nc.gpsimd.affine_select(
                ident[:], onesq[:], pattern=[[-1, 128]],
                compare_op=Alu.is_equal, fill=0.0, base=0, channel_multiplier=1,
            )
            iota_i = const.tile([128, TC_], i32, tag="iota_i")
            nc.gpsimd.iota(iota_i[:], pattern=[[128, TC_]], base=0, channel_multiplier=1)
            iota_f = const.tile([128, TC_], f32, tag="iota_f")
            nc.vector.tensor_copy(iota_f[:], iota_i[:])

            b1c = const.tile([128, KC], f32, tag="b1c")
            for m in range(KC):
                nc.sync.dma_start(b1c[:, m : m + 1], b1_ap[128 * m : 128 * (m + 1), :])
            b2c = const.tile([C, 1], f32, tag="b2c")
            nc.sync.dma_start(b2c[:], b2_ap[:])

            # weights: w1 row-chunks [128, 512]; rows 0..511 = W1a, 512..1023 = W1b
            w1sb = []
            for k in range(2 * KC):
                t = wpool.tile([128, D], bf16, tag=f"w1_{k}")
                nc.sync.dma_start(t[:], w1_ap[128 * k : 128 * (k + 1), :])
                w1sb.append(t)
            w2c = []
            for m in range(KC):
                t = wpool.tile([128, C], bf16, tag=f"w2_{m}")
                nc.sync.dma_start(t[:], w2_ap[128 * m : 128 * (m + 1), :])
                w2c.append(t)

            def body():
                # ---- load hidden rows ----
                ht = []
                for t_ in range(TC_):
                    tl = gpool.tile([128, D], bf16, tag=f"ht_{t_}")
                    nc.sync.dma_start(tl[:], h_ap[128 * t_ : 128 * (t_ + 1), :])
                    ht.append(tl)

                # ---- phase A: one-hot of idx, [t, n] layout ----
                idx_sb = gpool.tile([1, N], f32, tag="idx_sb")
                nc.sync.dma_start(idx_sb[:], idx_ap[:])
                ps_i = po.tile([128, N], f32, tag="po")
                nc.tensor.matmul(ps_i[:], ones1[:], idx_sb[:])  # bcast idx to 128 parts
                idxb = gpool.tile([128, N], f32, tag="idxb")
                nc.vector.tensor_copy(idxb[:], ps_i[:])
                oh = []
                for t_ in range(TC_):
                    o = gpool.tile([128, N], bf16, tag=f"oh_{t_}")
                    nc.vector.tensor_scalar(
                        o[:], idxb[:], iota_f[:, t_ : t_ + 1], None, op0=Alu.is_equal
                    )
                    oh.append(o)

                # ---- phase B: gather A_T = H_T @ onehot ----
                # |d| = 2*relu(d) - d : the per-pair matmul only needs
                # rp = relu(2*x_j - 2*x_i) (valid subtract+max dual op); the
                # linear -d part folds into Qmat = A @ W1b rank-one terms.
                at_bf, at2_bf, at32_2 = [], [], []
                for m in range(KC):
                    ps_g = po.tile([128, N], f32, tag="po")
                    for t_ in range(TC_):
                        nc.tensor.matmul(
                            ps_g[:],
                            ht[t_][:, 128 * m : 128 * (m + 1)],
                            oh[t_][:],
                            start=(t_ == 0),
                            stop=(t_ == TC_ - 1),
                        )
                    a_bf = gpool.tile([128, N], bf16, tag=f"at_bf_{m}")
                    nc.vector.tensor_copy(a_bf[:], ps_g[:])
                    # doubled [A|A] so cyclic column windows are contiguous
                    a2_bf = gpool.tile([128, 2 * N], bf16, tag=f"at2x_{m}")
                    nc.vector.tensor_scalar(
                        a2_bf[:, 0:N], ps_g[:], 2.0, None, op0=Alu.mult
                    )
                    nc.vector.tensor_scalar(
                        a2_bf[:, N : 2 * N], ps_g[:], 2.0, None, op0=Alu.mult
                    )
                    a2_32 = gpool.tile([128, NL], f32, tag=f"at32_2_{m}")
                    nc.vector.tensor_scalar(
                        a2_32[:], ps_g[:, 0:NL], 2.0, None, op0=Alu.mult
                    )
                    at_bf.append(a_bf)
                    at2_bf.append(a2_bf)
                    at32_2.append(a2_32)

                # ---- phase C: Pmat = A@W1a, Qmat = A@W1b (transposed layouts) ----
                pm32, qm32 = [], []
                for m in range(KC):
                    ps_p = po.tile([128, N], f32, tag="po")
                    for k in range(KC):
                        nc.tensor.matmul(
                            ps_p[:],
                            w1sb[k][:, 128 * m : 128 * (m + 1)],
                            at_bf[k][:],
                            start=(k == 0),
                            stop=(k == KC - 1),
                        )
                    p_32 = gpool.tile([128, N], f32, tag=f"pm32_{m}")
                    nc.vector.tensor_copy(p_32[:], ps_p[:])
                    pm32.append(p_32)
                for m in range(KC):
                    ps_q = po.tile([128, N], f32, tag="po")
                    for k in range(KC):
                        nc.tensor.matmul(
                            ps_q[:],
                            w1sb[KC + k][:, 128 * m : 128 * (m + 1)],
                            at_bf[k][:],
                            start=(k == 0),
                            stop=(k == KC - 1),
                        )
                    q_32 = gpool.tile([128, N], f32, tag=f"qm32_{m}")
                    nc.vector.tensor_copy(q_32[:], ps_q[:])
                    qm32.append(q_32)
                # PmQ = Pmat - Qmat (j-term), PpQb = Pmat + Qmat + b1 (i-term)
                pmq_bf, ppqb32 = [], []
                for m in range(KC):
                    d_bf = gpool.tile([128, 2 * N], bf16, tag=f"pmq2x_{m}")
                    nc.vector.tensor_tensor(
                        d_bf[:, 0:N], pm32[m][:], qm32[m][:], op=Alu.subtract
                    )
                    nc.vector.tensor_tensor(
                        d_bf[:, N : 2 * N], pm32[m][:], qm32[m][:], op=Alu.subtract
                    )
                    s_32 = gpool.tile([128, NL], f32, tag=f"ppqb32_{m}")
                    nc.vector.scalar_tensor_tensor(
                        s_32[:],
                        pm32[m][:, 0:NL],
                        b1c[:, m : m + 1],
                        qm32[m][:, 0:NL],
                        op0=Alu.add,
                        op1=Alu.add,
                    )
                    pmq_bf.append(d_bf)
                    ppqb32.append(s_32)

                # ---- main loop over row-quads: row i covers cyclic cols
                # j = i..i+127 (the symmetric half of the grid) ----
                for q in range(NQ):
                    absq = work.tile([128, KC * QR * TW], bf16, tag="absq")
                    pp = work.tile([128, KC * QR * TW], bf16, tag="pp")
                    for k in range(KC):
                        for r in range(QR):
                            i = QR * q + r
                            nc.vector.tensor_scalar(
                                absq[:, 512 * k + TW * r : 512 * k + TW * (r + 1)],
                                at2_bf[k][:, i : i + TW],
                                at32_2[k][:, i : i + 1],
                                0.0,
                                op0=Alu.subtract,
                                op1=Alu.max,
                            )
                    for m in range(KC):
                        for r in range(QR):
                            i = QR * q + r
                            nc.vector.tensor_scalar(
                                pp[:, 512 * m + TW * r : 512 * m + TW * (r + 1)],
                                pmq_bf[m][:, i : i + TW],
                                ppqb32[m][:, i : i + 1],
                                None,
                                op0=Alu.add,
                            )

                    hh = work.tile([128, KC * 512], bf16, tag="hh")
                    for m in range(KC):
                        ps_h = ph.tile([128, 512], f32, tag="ph")
                        # P/Q rank-one terms via identity weights; start=True
                        # zeroes the 2KB bank region
                        nc.tensor.matmul(
                            ps_h[:],
                            ident[:],
                            pp[:, 512 * m : 512 * (m + 1)],
                            start=True,
                            stop=False,
                        )
                        for k in range(KC):
                            nc.tensor.matmul(
                                ps_h[:],
                                w1sb[KC + k][:, 128 * m : 128 * (m + 1)],
                                absq[:, 512 * k : 512 * (k + 1)],
                                start=False,
                                stop=(k == KC - 1),
                            )
                        nc.scalar.activation(
                            hh[:, 512 * m : 512 * (m + 1)], ps_h[:], Act.Gelu
                        )

                    ps_o = po.tile([C, 512], f32, tag="po")
                    for m in range(KC):
                        nc.tensor.matmul(
                            ps_o[:],
                            w2c[m][:],
                            hh[:, 512 * m : 512 * (m + 1)],
                            start=(m == 0),
                            stop=(m == KC - 1),
                        )
                    tmp = opool.tile([C, 512], f32, tag="tmp")
                    nc.vector.tensor_scalar(
                        tmp[:], ps_o[:], b2c[:], None, op0=Alu.add
                    )
                    outq = opool.tile([C, 512], f32, tag="outq")
                    # col t==0 of each row block is j==i: the diagonal
                    nc.gpsimd.affine_select(
                        outq[:], tmp[:], pattern=[[0, QR], [1, TW]],
                        compare_op=Alu.not_equal, fill=MASK_FILL,
                        base=0, channel_multiplier=0,
                    )
                    nc.sync.dma_start(out_ap[:, QR * q : QR * (q + 1), 0:TW], outq[:])

                # ---- antipodal pass: pairs (i, i+128), offset not covered
                # by the half-window nor by the mirror ----
                rpA = work.tile([128, KC * NL], bf16, tag="rpA")
                ppA = work.tile([128, KC * NL], bf16, tag="ppA")
                for k in range(KC):
                    dA = work.tile([128, NL], f32, tag="dA")
                    nc.vector.tensor_tensor(
                        dA[:], at2_bf[k][:, TW : TW + NL], at2_bf[k][:, 0:NL],
                        op=Alu.subtract,
                    )
                    nc.vector.tensor_scalar(
                        rpA[:, NL * k : NL * (k + 1)], dA[:], 0.0, None, op0=Alu.max
                    )
                for m in range(KC):
                    nc.vector.tensor_tensor(
                        ppA[:, NL * m : NL * (m + 1)],
                        pmq_bf[m][:, TW : TW + NL],
                        ppqb32[m][:],
                        op=Alu.add,
                    )
                hhA = work.tile([128, KC * NL], bf16, tag="hhA")
                for m in range(KC):
                    psA = ph.tile([128, NL], f32, tag="ph")
                    nc.tensor.matmul(
                        psA[:], ident[:], ppA[:, NL * m : NL * (m + 1)],
                        start=True, stop=False,
                    )
                    for k in range(KC):
                        nc.tensor.matmul(
                            psA[:],
                            w1sb[KC + k][:, 128 * m : 128 * (m + 1)],
                            rpA[:, NL * k : NL * (k + 1)],
                            start=False,
                            stop=(k == KC - 1),
                        )
                    nc.scalar.activation(
                        hhA[:, NL * m : NL * (m + 1)], psA[:], Act.Gelu
                    )
                psoA = po.tile([C, NL], f32, tag="po")
                for m in range(KC):
                    nc.tensor.matmul(
                        psoA[:], w2c[m][:], hhA[:, NL * m : NL * (m + 1)],
                        start=(m == 0), stop=(m == KC - 1),
                    )
                tmpA = opool.tile([C, NL], f32, tag="tmpA")
                nc.vector.tensor_scalar(tmpA[:], psoA[:], b2c[:], None, op0=Alu.add)
                nc.sync.dma_start(out_ap[:, :, TW : TW + 1], tmpA[:])

            for _ in range(reps):
                body()

    nc.compile()
    return nc


def _get(reps=1):
    if reps not in _CACHE:
        _CACHE[reps] = _build(reps)
    return _CACHE[reps]


def _shard_inputs(hidden_states, W1, b1, W2, b2, atom_indices):
    hs = np.asarray(hidden_states, np.float32)
    idx = np.clip(np.asarray(atom_indices).astype(np.int64), 0, T - 1)
    w1b = np.asarray(W1, np.float32).astype(ml_dtypes.bfloat16)
    w2b = np.asarray(W2, np.float32).astype(ml_dtypes.bfloat16)
    b1f = np.asarray(b1, np.float32).reshape(D, 1)
    b2f = np.asarray(b2, np.float32).reshape(C, 1)
    in_maps = []
    for c in range(NCORES):
        b = c // RB
        r0 = NL * (c % RB)
        idx_roll = np.roll(idx[b], -r0).astype(np.float32).reshape(1, N)
        in_maps.append(
            {
                "h": hs[b].astype(ml_dtypes.bfloat16),
                "idxf": idx_roll,
                "w1": w1b,
                "w2": w2b,
                "b1": b1f,
                "b2": b2f,
            }
        )
    return in_maps


def _unshard(results, atom_mask):
    full = np.empty((B, C, N, N), np.float32)
    for c in range(NCORES):
        b = c // RB
        r0 = NL * (c % RB)
        blk = results[c]["out"]  # [C, 64, 129]: row i -> cols (i+t)%N, t=0..128
        rows = r0 + np.arange(NL)
        idx_j = (rows[:, None] + np.arange(TW + 1)[None, :]) % N  # [64, 129]
        np.put_along_axis(
            full[b, :, r0 : r0 + NL, :],
            np.broadcast_to(idx_j[None], (C, NL, TW + 1)),
            blk,
            axis=2,
        )
    # grid symmetry: logits[i,j] == logits[j,i]; offsets 129..255 mirror
    offs = (np.arange(N)[None, :] - np.arange(N)[:, None]) % N
    low = offs > TW
    fullT = np.transpose(full, (0, 1, 3, 2))
    full = np.where(low[None, None], fullT, full)
    mask = np.asarray(atom_mask).astype(bool)
    if not mask.all():
        valid = mask[:, :, None] & mask[:, None, :]
        valid &= ~np.eye(N, dtype=bool)[None]
        full = np.where(valid[:, None, :, :], full, np.float32(MASK_FILL))
    return full


def kernel(hidden_states, W1, b1, W2, b2, atom_indices, atom_mask):
    from concourse.bass_utils import run_bass_kernel_spmd

    nc = _get(1)
    in_maps = _shard_inputs(hidden_states, W1, b1, W2, b2, atom_indices)
    res = run_bass_kernel_spmd(nc, in_maps, list(range(NCORES)))
    return _unshard(res.results, atom_mask)

